# revision 9
# baseline (speedup 1.0000x reference)
"""Transformer decoder layer (causal self-attn + cross-attn + FFN, post-LN)
on 8 trn2 NeuronCores via Bass/Tile.

Sharding (core c = 4*b + j; b = batch, j = rank in the 4-core batch group):
  - self-attention: HEAD-sharded (4 heads/core, all 2048 tokens, causal).
  - wo after self-attn: computed for ALL tokens against this core's 256
    head-dims, then ReduceScatter(add) within the batch group.
  - everything else (LN, cross-attn queries/output, FFN): TOKEN-sharded.
  - cross-attn K/V: each core projects its 512-token slice of `encoding`;
    AllGather within the batch group.

Perf structure (vs the first working version):
  - All DRAM tensors are host-pre-tiled to partition-major [128, ...]
    layouts so every load is a few 128-descriptor DMAs.
  - Cross K/V projections are emitted AFTER self-attention so the tensor
    engine has work while the ReduceScatter runs.
  - Softmax normalize uses gpsimd.partition_broadcast (no DRAM bounce);
    av PSUM tiles are double-buffered so the next head's AV stream never
    waits on the previous head's normalize chain.
  - LayerNorm: inverse std via one Rsqrt activation, row broadcasts on
    gpsimd, per-chunk normalize split across vector+gpsimd with the
    gamma/beta affine fused into the scalar-engine downcast.
  - Residuals kept in SBUF as bf16 (no f32 DRAM bounce).
  - FFN1 eviction is a fused bias+ReLU on the scalar engine.
"""
import os
import numpy as np
import ml_dtypes

import concourse.bass as bass
import concourse.mybir as mybir
import concourse.tile as tile
from concourse import bacc
from concourse.bass_utils import run_bass_kernel_spmd

F32 = mybir.dt.float32
BF16 = mybir.dt.bfloat16
AF = mybir.ActivationFunctionType
OP = mybir.AluOpType

B, S, D, DHID, H = 2, 2048, 1024, 4096, 16
NT = 512
HL = 4
EPS = 1e-6
SCALE = 1.0 / 32.0

_CACHE = {}
LAST_RESULT = None


def _bf(a):
    return np.ascontiguousarray(np.asarray(a).astype(ml_dtypes.bfloat16))


def _f32(a):
    return np.ascontiguousarray(np.asarray(a, dtype=np.float32))


def build_nc():
    nc = bacc.Bacc("TRN2", target_bir_lowering=False, debug=False, num_devices=8)

    def inp(name, shape, dt=BF16):
        return nc.dram_tensor(name, shape, dt, kind="ExternalInput").ap()

    # all inputs pre-tiled partition-major on host
    xtf = inp("xtf", [128, 8, 2048])          # x^T (full batch row), d-chunked
    xsl = inp("xsl", [128, 8, 512])           # x^T token slice (this core)
    ekv = inp("ekv", [128, 8, 512])           # enc^T token slice
    wq_blk = inp("wq_blk", [128, 8, 256])
    wk_blk = inp("wk_blk", [128, 8, 256])
    wv_blk = inp("wv_blk", [128, 8, 256])
    wo_blk = inp("wo_blk", [128, 2, 1024])
    wqTc = inp("wqTc", [128, 8, 1024])
    wkTc = inp("wkTc", [128, 8, 1024])
    wvTc = inp("wvTc", [128, 8, 1024])
    woTc = inp("woTc", [128, 8, 1024])
    w1T = inp("w1T", [8, 128, 8, 512])        # hg-chunked
    w2T = inp("w2T", [8, 128, 32, 128])       # jt-chunked
    b1v = inp("b1v", [128, 32], F32)
    b2v = inp("b2v", [128, 8], F32)
    lng = inp("lng", [128, 3, 8], F32)
    lnb = inp("lnb", [128, 3, 8], F32)
    masks = inp("masks", [128, 4, 512])
    out_d = nc.dram_tensor("out", [128, 8, 512], F32, kind="ExternalOutput").ap()

    RG = [[0, 1, 2, 3], [4, 5, 6, 7]]

    with tile.TileContext(nc) as tc:
        with (
            tc.tile_pool(name="ps_sc", bufs=3, space="PSUM") as ps_sc,
            tc.tile_pool(name="ps_av", bufs=2, space="PSUM") as ps_av,
            tc.tile_pool(name="dram", bufs=1, space="DRAM") as dram,
            tc.tile_pool(name="pers", bufs=1) as pers,
            tc.tile_pool(name="wts", bufs=2) as wts,
            tc.tile_pool(name="wrk", bufs=2) as wrk,
            tc.tile_pool(name="expool", bufs=2) as expool,
            tc.tile_pool(name="rws", bufs=2) as rws,
        ):
            # ---------- static small sbuf ----------
            ones1 = pers.tile([128, 1], BF16, tag="ones1")
            nc.vector.memset(ones1[:], 1.0)
            mask_sb = pers.tile([128, 4, 512], BF16, tag="mask")
            nc.sync.dma_start(mask_sb[:], masks)
            g_sb = pers.tile([128, 3, 8], F32, tag="lng")
            nc.sync.dma_start(g_sb[:], lng)
            bta_sb = pers.tile([128, 3, 8], F32, tag="lnb")
            nc.sync.dma_start(bta_sb[:], lnb)
            b1_sb = pers.tile([128, 32], F32, tag="b1")
            nc.sync.dma_start(b1_sb[:], b1v)
            b2_sb = pers.tile([128, 8], F32, tag="b2")
            nc.sync.dma_start(b2_sb[:], b2v)

            # ---------- phase B inputs ----------
            xtf_sb = pers.tile([128, 8, 2048], BF16, tag="big32")
            for kt in range(8):
                nc.sync.dma_start(xtf_sb[:, kt, :], xtf[:, kt, :])
            ekv_sb = pers.tile([128, 8, 512], BF16, tag="ekv8")
            nc.sync.dma_start(ekv_sb[:], ekv)
            wqb = pers.tile([128, 8, 256], BF16, tag="wblk", bufs=2)
            nc.sync.dma_start(wqb[:], wq_blk)
            wkb = pers.tile([128, 8, 256], BF16, tag="wblk", bufs=2)
            nc.sync.dma_start(wkb[:], wk_blk)
            wob = pers.tile([128, 2, 1024], BF16, tag="wob")
            nc.sync.dma_start(wob[:], wo_blk)
            # cross weights prefetched into the streaming ring (DMAs run
            # during self-attention; compute consumes them much later)
            wk_c = wts.tile([128, 8, 1024], BF16, tag="w16")
            nc.sync.dma_start(wk_c[:], wkTc)
            wv_c = wts.tile([128, 8, 1024], BF16, tag="w16")
            nc.sync.dma_start(wv_c[:], wvTc)

            # ---------- phase B: self QKV (head-block) ----------
            qt_s = pers.tile([128, 2, 2048], BF16, tag="qt8")
            kt_s = pers.tile([128, 2, 2048], BF16, tag="kb8")
            for jt in range(2):
                for dst, w in ((qt_s, wqb), (kt_s, wkb)):
                    for tw in range(4):
                        pt = ps_sc.tile([128, 512], F32, tag="sc")
                        for kt in range(8):
                            nc.tensor.matmul(
                                pt[:], w[:, kt, 128 * jt:128 * jt + 128],
                                xtf_sb[:, kt, 512 * tw:512 * tw + 512],
                                start=(kt == 0), stop=(kt == 7))
                        nc.any.tensor_copy(
                            dst[:, jt, 512 * tw:512 * tw + 512], pt[:])
            wvb = pers.tile([128, 8, 256], BF16, tag="wblk", bufs=2)
            nc.sync.dma_start(wvb[:], wv_blk)
            vhat_s = pers.tile([128, 16, HL, 65], BF16, tag="vh8")
            nc.vector.memset(vhat_s[:, :, :, 64:65], 1.0)
            for tt in range(16):
                pt = ps_sc.tile([128, 512], F32, tag="sc")
                for kt in range(8):
                    nc.tensor.matmul(
                        pt[:, 0:256], xtf_sb[:, kt, 128 * tt:128 * tt + 128],
                        wvb[:, kt, :], start=(kt == 0), stop=(kt == 7))
                nc.any.tensor_copy(
                    vhat_s[:, tt, :, 0:64],
                    pt[:, 0:256].rearrange("p (h d) -> p h d", h=HL))

            # ---------- collectives' DRAM buffers ----------
            rs_in = dram.tile([4, 128, 8, 512], BF16)
            rs_out = dram.tile([128, 8, 512], BF16)
            ag_in = dram.tile([2, 1024, 512], BF16)
            ag_in0 = ag_in[0]
            ag_inV = ag_in[1].rearrange("a t -> (a t)").rearrange(
                "(q p tt hh dd) -> q p tt hh dd", q=8, p=128, tt=4, hh=2)
            ag_out = dram.tile([4, 2, 1024, 512], BF16)

            def softmax_norm(av, attn_dst):
                """attn_dst <- av[0:64]/av[64] (row-broadcast divide)."""
                den = rws.tile([1, 512], F32, tag="row", bufs=3)
                nc.vector.tensor_copy(den[:], av[64:65, :])
                rec = rws.tile([1, 512], F32, tag="row", bufs=3)
                nc.vector.reciprocal_approx_fast(rec[:], den[:])
                recR = rws.tile([64, 512], F32, tag="recR", bufs=2)
                nc.gpsimd.partition_broadcast(recR[:], rec[:])
                nc.vector.tensor_tensor(attn_dst, av[0:64, :], recR[:], OP.mult)

            # ---------- phase C: self-attention ----------
            attnT = pers.tile([128, 2, 2048], BF16, tag="atS")

            def wo_partial(tc_):
                for jt in range(8):
                    pt = ps_sc.tile([128, 512], F32, tag="sc")
                    for kt in range(2):
                        nc.tensor.matmul(
                            pt[:], wob[:, kt, 128 * jt:128 * jt + 128],
                            attnT[:, kt, 512 * tc_:512 * tc_ + 512],
                            start=(kt == 0), stop=(kt == 1))
                    ws = wrk.tile([128, 512], BF16, tag="wocp")
                    nc.vector.tensor_copy(ws[:], pt[:])
                    nc.sync.dma_start(rs_in[tc_, :, jt, :], ws[:])

            for qc in range(4):
                nkt = 4 * (qc + 1)
                for p in range(2):
                    for m in range(2):
                        p0 = 64 * m
                        av = ps_av.tile([65, 512], F32, tag="av")
                        for g in range(nkt // 2):
                            sc = ps_sc.tile([128, 2, 512], F32, tag="sc")
                            for i in range(2):
                                kt = 2 * g + i
                                nc.tensor.matmul(
                                    sc[:, i, :],
                                    kt_s[p0:p0 + 64, p, 128 * kt:128 * kt + 128],
                                    qt_s[p0:p0 + 64, p, 512 * qc:512 * qc + 512],
                                    start=True, stop=True)
                            ex = expool.tile([128, 2, 512], BF16, tag="ex")
                            nc.scalar.activation(ex[:], sc[:], AF.Exp, scale=SCALE)
                            for i in range(2):
                                r = 2 * g + i - (nkt - 4)
                                if 0 <= r < 4:
                                    nc.vector.tensor_tensor(
                                        ex[:, i, :], ex[:, i, :],
                                        mask_sb[:, r, :], OP.mult)
                            for i in range(2):
                                kt = 2 * g + i
                                nc.tensor.matmul(
                                    av[:], vhat_s[:, kt, 2 * p + m, :],
                                    ex[:, i, :],
                                    start=(kt == 0), stop=(kt == nkt - 1))
                        softmax_norm(
                            av, attnT[p0:p0 + 64, p, 512 * qc:512 * qc + 512])
                    if p == 0 and qc > 0:
                        wo_partial(qc - 1)  # deferred: inputs ready, PE stays fed

            # ---------- phase A1: cross K proj (fills RS shadow) ----------
            for jt in range(8):
                pt = ps_sc.tile([128, 512], F32, tag="sc")
                for kt in range(8):
                    nc.tensor.matmul(
                        pt[:], wk_c[:, kt, 128 * jt:128 * jt + 128],
                        ekv_sb[:, kt, :], start=(kt == 0), stop=(kt == 7))
                kc = wrk.tile([128, 512], BF16, tag="wocp")
                nc.scalar.activation(kc[:], pt[:], AF.Copy)
                nc.sync.dma_start(ag_in0[128 * jt:128 * jt + 128, :], kc[:])

            wo_partial(3)
            nc.gpsimd.collective_compute(
                "ReduceScatter", OP.add, replica_groups=RG,
                ins=[rs_in[:].opt()], outs=[rs_out[:].opt()])

            # cross Q / self wo weights into the ring while RS runs
            wq_c = wts.tile([128, 8, 1024], BF16, tag="w16")
            nc.sync.dma_start(wq_c[:], wqTc)
            wo_c = wts.tile([128, 8, 1024], BF16, tag="w16")
            nc.sync.dma_start(wo_c[:], woTc)

            # ---------- phase A2: cross V proj (pair-major for AG) ----------
            for tt in range(4):
                for s in range(2):
                    pt = ps_sc.tile([128, 512], F32, tag="sc")
                    for kt in range(8):
                        nc.tensor.matmul(
                            pt[:], ekv_sb[:, kt, 128 * tt:128 * tt + 128],
                            wv_c[:, kt, 512 * s:512 * s + 512],
                            start=(kt == 0), stop=(kt == 7))
                    vc = wrk.tile([128, 512], BF16, tag="wocp")
                    nc.scalar.activation(vc[:], pt[:], AF.Copy)
                    for k in range(4):
                        nc.sync.dma_start(
                            ag_inV[4 * s + k, :, tt, :, :],
                            vc[:, 128 * k:128 * k + 128].rearrange(
                                "p (hh dd) -> p hh dd", hh=2))

            # ---------- phase E: resid1 + LN1 ----------
            xsl_sb = wrk.tile([128, 8, 512], BF16, tag="tmp8")
            nc.sync.dma_start(xsl_sb[:], xsl)
            sa_tok = wrk.tile([128, 8, 512], BF16, tag="tmp8")
            nc.sync.dma_start(sa_tok[:], rs_out[:])
            resid1 = pers.tile([128, 8, 512], BF16, tag="rsd")
            nc.vector.tensor_tensor(resid1[:], xsl_sb[:], sa_tok[:], OP.add)

            def layernorm(src, ln_idx, out_bf, out_f32_dram, src_bf=None):
                if src_bf is None:
                    src_bf = src
                sq = wrk.tile([128, 8, 512], BF16, tag="tmp8")
                nc.vector.tensor_tensor(sq[:], src_bf[:], src_bf[:], OP.mult)
                psum = ps_av.tile([1, 512], F32, tag="av")
                psq = ps_av.tile([1, 512], F32, tag="av")
                for kt in range(8):
                    nc.tensor.matmul(psum[:], ones1[:], src_bf[:, kt, :],
                                     start=(kt == 0), stop=(kt == 7))
                for kt in range(8):
                    nc.tensor.matmul(psq[:], ones1[:], sq[:, kt, :],
                                     start=(kt == 0), stop=(kt == 7))
                mean = rws.tile([1, 512], F32, tag="row", bufs=3)
                nc.vector.tensor_scalar(mean[:], psum[:], 1.0 / D, None, OP.mult)
                var = rws.tile([1, 512], F32, tag="row", bufs=3)
                nc.vector.tensor_tensor(var[:], psum[:], mean[:], OP.mult)
                nc.vector.tensor_tensor(var[:], psq[:], var[:], OP.subtract)
                nc.vector.tensor_scalar(var[:], var[:], 1.0 / (D - 1), None,
                                        OP.mult)
                std = rws.tile([1, 512], F32, tag="row", bufs=3)
                nc.scalar.activation(std[:], var[:], AF.Sqrt)
                nc.vector.tensor_scalar(std[:], std[:], EPS, None, OP.add)
                r_row = rws.tile([1, 512], F32, tag="row", bufs=3)
                nc.vector.reciprocal_approx_fast(r_row[:], std[:])
                mr = rws.tile([1, 512], F32, tag="row", bufs=3)
                nc.vector.tensor_tensor(mr[:], mean[:], r_row[:], OP.mult)
                rR = rws.tile([128, 512], F32, tag="rR", bufs=1)
                nc.gpsimd.partition_broadcast(rR[:], r_row[:])
                mR = rws.tile([128, 512], F32, tag="mR", bufs=1)
                nc.gpsimd.partition_broadcast(mR[:], mr[:])
                for dt in range(8):
                    eng = nc.vector if dt < 5 else nc.gpsimd
                    t1 = wrk.tile([128, 512], F32, tag="lnt", bufs=3)
                    eng.tensor_tensor(t1[:], src[:, dt, :], rR[:], OP.mult)
                    eng.tensor_tensor(t1[:], t1[:], mR[:], OP.subtract)
                    gcol = g_sb[:, ln_idx, dt:dt + 1]
                    bcol = bta_sb[:, ln_idx, dt:dt + 1]
                    if out_f32_dram is not None:
                        of = wrk.tile([128, 512], F32, tag="outp")
                        nc.scalar.activation(of[:], t1[:], AF.Identity,
                                             bias=bcol, scale=gcol)
                        nc.sync.dma_start(out_f32_dram[:, dt, :], of[:])
                    else:
                        nc.scalar.activation(out_bf[:, dt, :], t1[:],
                                             AF.Identity, bias=bcol,
                                             scale=gcol)

            h1b = pers.tile([128, 8, 512], BF16, tag="kb8")
            layernorm(resid1, 0, h1b, None)

            # AG after LN1's gpsimd ops so they aren't queued behind it
            nc.gpsimd.collective_compute(
                "AllGather", OP.bypass, replica_groups=RG,
                ins=[ag_in[:].opt()], outs=[ag_out[:].opt()])

            ag_outV = ag_out[:, 1].rearrange("r a t -> r (a t)").rearrange(
                "r (q p tt hh dd) -> r q p tt hh dd", q=8, p=128, tt=4, hh=2)

            def cross_load(p):
                ktp = wrk.tile([128, 2048], BF16, tag="ktp")
                for r in range(4):
                    nc.sync.dma_start(
                        ktp[:, 512 * r:512 * r + 512],
                        ag_out[r, 0, 128 * p:128 * p + 128, :])
                vhp = wrk.tile([128, 16, 2, 65], BF16, tag="vhp")
                nc.vector.memset(vhp[:, :, :, 64:65], 1.0)
                for r in range(4):
                    nc.sync.dma_start(
                        vhp[:, 4 * r:4 * r + 4, :, 0:64], ag_outV[r, p])
                return ktp, vhp

            cross_tiles = {0: cross_load(0), 1: cross_load(1)}

            # ---------- phase F: cross Q ----------
            qt_c = pers.tile([128, 8, 512], BF16, tag="qt8")
            for jt in range(8):
                pt = ps_sc.tile([128, 512], F32, tag="sc")
                for kt in range(8):
                    nc.tensor.matmul(
                        pt[:], wq_c[:, kt, 128 * jt:128 * jt + 128],
                        h1b[:, kt, :], start=(kt == 0), stop=(kt == 7))
                nc.any.tensor_copy(qt_c[:, jt, :], pt[:])

            # ---------- phase G: cross-attention ----------
            attnT2 = pers.tile([128, 8, 512], BF16, tag="atS")
            for p in range(8):
                ktp, vhp = cross_tiles.pop(p)
                if p + 2 < 8:
                    cross_tiles[p + 2] = cross_load(p + 2)
                for m in range(2):
                    p0 = 64 * m
                    av = ps_av.tile([65, 512], F32, tag="av")
                    for g in range(8):
                        sc = ps_sc.tile([128, 2, 512], F32, tag="sc")
                        for i in range(2):
                            kt = 2 * g + i
                            nc.tensor.matmul(
                                sc[:, i, :],
                                ktp[p0:p0 + 64, 128 * kt:128 * kt + 128],
                                qt_c[p0:p0 + 64, p, :],
                                start=True, stop=True)
                        ex = expool.tile([128, 2, 512], BF16, tag="ex")
                        nc.scalar.activation(ex[:], sc[:], AF.Exp, scale=SCALE)
                        for i in range(2):
                            kt = 2 * g + i
                            nc.tensor.matmul(
                                av[:], vhp[:, kt, m, :], ex[:, i, :],
                                start=(kt == 0), stop=(kt == 15))
                    softmax_norm(av, attnT2[p0:p0 + 64, p, :])

            # ---------- phase H: cross wo + resid2 + LN2 ----------
            resid2 = pers.tile([128, 8, 512], BF16, tag="rsd")
            for jt in range(8):
                pt = ps_sc.tile([128, 512], F32, tag="sc")
                for kt in range(8):
                    nc.tensor.matmul(
                        pt[:], wo_c[:, kt, 128 * jt:128 * jt + 128],
                        attnT2[:, kt, :], start=(kt == 0), stop=(kt == 7))
                nc.vector.tensor_tensor(resid2[:, jt, :], pt[:], h1b[:, jt, :],
                                        OP.add)
            h2b = pers.tile([128, 8, 512], BF16, tag="vh8")
            layernorm(resid2, 1, h2b, None)

            # ---------- phase I: FFN + resid3 + LN3 -> out ----------
            w1_tiles = {}
            for hg in range(2):
                w1_tiles[hg] = wts.tile([128, 8, 512], BF16, tag="w16", name=f"w1c{hg}")
                nc.sync.dma_start(w1_tiles[hg][:], w1T[hg])
            zrelu = pers.tile([128, 32, 512], BF16, tag="big32")
            for hg in range(8):
                w1_sb = w1_tiles.pop(hg)
                if hg + 2 < 8:
                    w1_tiles[hg + 2] = wts.tile([128, 8, 512], BF16, tag="w16", name=f"w1c{hg + 2}")
                    nc.sync.dma_start(w1_tiles[hg + 2][:], w1T[hg + 2])
                for hh in range(4):
                    ht = 4 * hg + hh
                    pt = ps_sc.tile([128, 512], F32, tag="sc")
                    for kt in range(8):
                        nc.tensor.matmul(
                            pt[:], w1_sb[:, kt, 128 * hh:128 * hh + 128],
                            h2b[:, kt, :], start=(kt == 0), stop=(kt == 7))
                    nc.scalar.activation(zrelu[:, ht, :], pt[:], AF.Relu,
                                         bias=b1_sb[:, ht:ht + 1])

            resid3 = pers.tile([128, 8, 512], F32, tag="rsd")
            w2_tiles = {}
            for jt in range(2):
                w2_tiles[jt] = wrk.tile([128, 32, 128], BF16, tag="ktp", name=f"w2c{jt}")
                nc.sync.dma_start(w2_tiles[jt][:], w2T[jt])
            for jt in range(8):
                w2_sb = w2_tiles.pop(jt)
                if jt + 2 < 8:
                    w2_tiles[jt + 2] = wrk.tile([128, 32, 128], BF16, tag="ktp", name=f"w2c{jt + 2}")
                    nc.sync.dma_start(w2_tiles[jt + 2][:], w2T[jt + 2])
                pt = ps_sc.tile([128, 512], F32, tag="sc")
                for kt in range(32):
                    nc.tensor.matmul(
                        pt[:], w2_sb[:, kt, :], zrelu[:, kt, :],
                        start=(kt == 0), stop=(kt == 31))
                s1 = wrk.tile([128, 512], F32, tag="outp")
                nc.scalar.activation(s1[:], pt[:], AF.Identity,
                                     bias=b2_sb[:, jt:jt + 1])
                nc.vector.tensor_tensor(resid3[:, jt, :], s1[:], h2b[:, jt, :],
                                        OP.add)
            r3b = wrk.tile([128, 8, 512], BF16, tag="tmp8")
            nc.vector.tensor_copy(r3b[:], resid3[:])
            layernorm(resid3, 2, None, out_d, src_bf=r3b)

    nc.compile()
    return nc


def _host_prep(inputs):
    x = _f32(inputs["x"])
    enc = _f32(inputs["encoding"])
    wT = {k: _bf(np.asarray(inputs[k]).T) for k in
          ("sa_wq", "sa_wk", "sa_wv", "sa_wo", "ca_wq", "ca_wk", "ca_wv",
           "ca_wo", "ff_w1", "ff_w2")}

    def ptile(a, nk):
        # [nk*128, j] -> [128, nk, j]
        a = np.asarray(a)
        return np.ascontiguousarray(
            a.reshape(nk, 128, a.shape[-1]).transpose(1, 0, 2))

    lng = np.stack([_f32(inputs["ln1_g"]), _f32(inputs["ln2_g"]),
                    _f32(inputs["ln3_g"])])          # [3, 1024]
    lnb = np.stack([_f32(inputs["ln1_b"]), _f32(inputs["ln2_b"]),
                    _f32(inputs["ln3_b"])])
    lng_t = _f32(lng.reshape(3, 8, 128).transpose(2, 0, 1))   # [128, 3, 8]
    lnb_t = _f32(lnb.reshape(3, 8, 128).transpose(2, 0, 1))

    masks = np.zeros((4, 128, 512), np.float32)
    i = np.arange(128)[:, None]
    q = np.arange(512)[None, :]
    for r in range(4):
        masks[r] = (128 * r + i <= q).astype(np.float32)
    masks_t = _bf(masks.transpose(1, 0, 2))          # [128, 4, 512]

    w1c = np.stack([ptile(wT["ff_w1"][:, 512 * hg:512 * hg + 512], 8)
                    for hg in range(8)])             # [8, 128, 8, 512]
    w2c = np.stack([ptile(wT["ff_w2"][:, 128 * jt:128 * jt + 128], 32)
                    for jt in range(8)])             # [8, 128, 32, 128]
    b1t = _f32(np.asarray(inputs["ff_b1"]).reshape(32, 128).T)
    b2t = _f32(np.asarray(inputs["ff_b2"]).reshape(8, 128).T)

    wqc_t = ptile(wT["ca_wq"], 8)
    wkc_t = ptile(wT["ca_wk"], 8)
    wvc_t = ptile(wT["ca_wv"], 8)
    woc_t = ptile(wT["ca_wo"], 8)

    in_maps = []
    for c in range(8):
        b, j = c // 4, c % 4
        xT = _bf(x[b].T)                             # [1024, 2048]
        encT = _bf(enc[b].T)
        sl = slice(NT * j, NT * (j + 1))
        hb = slice(256 * j, 256 * (j + 1))
        in_maps.append({
            "xtf": ptile(xT, 8),
            "xsl": ptile(np.ascontiguousarray(xT[:, sl]), 8),
            "ekv": ptile(np.ascontiguousarray(encT[:, sl]), 8),
            "wq_blk": ptile(np.ascontiguousarray(wT["sa_wq"][:, hb]), 8),
            "wk_blk": ptile(np.ascontiguousarray(wT["sa_wk"][:, hb]), 8),
            "wv_blk": ptile(np.ascontiguousarray(wT["sa_wv"][:, hb]), 8),
            "wo_blk": ptile(np.ascontiguousarray(wT["sa_wo"][hb, :]), 2),
            "wqTc": wqc_t, "wkTc": wkc_t, "wvTc": wvc_t, "woTc": woc_t,
            "w1T": w1c, "w2T": w2c, "b1v": b1t, "b2v": b2t,
            "lng": lng_t, "lnb": lnb_t, "masks": masks_t,
        })
    return in_maps


def kernel(**inputs):
    global LAST_RESULT
    if "nc" not in _CACHE:
        _CACHE["nc"] = build_nc()
    nc = _CACHE["nc"]
    in_maps = _host_prep(inputs)
    res = run_bass_kernel_spmd(nc, in_maps, list(range(8)),
                               trace=bool(os.environ.get("BASS_TRACE")))
    LAST_RESULT = res
    out = np.zeros((B, S, D), np.float32)
    for c in range(8):
        b, j = c // 4, c % 4
        o = res.results[c]["out"]                    # [128, 8, 512]
        out[b, NT * j:NT * (j + 1), :] = (
            o.transpose(2, 1, 0).reshape(NT, D))
    return out
